# revision 33
# baseline (speedup 1.0000x reference)
"""Attention-pooling kernel for TRN2 (8 NeuronCores, data-parallel over batch).

Computes, per batch b:
    scores = seeds @ x[b].T          # [M, S]
    weights = softmax(scores, -1)
    out[b] = weights @ x[b]          # [M, D]

Sharding: batch B=32 split 4-per-core across 8 cores; seeds replicated.

Per-core pipeline (all bf16 on-chip, f32 PSUM accumulation):
  - SWDGE cast-DMA loads x tiles HBM f32 -> SBUF bf16 (cast rides the DMA).
  - PE transposes x 128x128 blocks (bf16, FWL weight loads) -> psum -> DVE
    copies to SBUF as x^T chunks.
  - scores: 4 accumulating matmuls lhsT=seedsT chunk [128,16], rhs=xT [128,512].
  - exp on ACT straight out of PSUM, with fused accum_out row-sums
    (no max subtraction: scores = seeds.x are bounded ~|8|, exp is safe in f32).
  - PE transposes exp [16,128] -> expT [128,16]; pooled matmuls are 4-way
    column-tiled (concurrent PE col-groups), partials accumulate in
    psum[32q:32q+16, :] over the whole batch.
  - batch end: reduce partials + recip(sum) on DVE, scale, DMA out f32.
  - Stages are software-pipelined (C(i-2), B(i-1), A(i)) so PE never waits
    on the ACT/DVE round trip of the same macro-tile.
"""

from contextlib import ExitStack

import numpy as np

import concourse.mybir as mybir
import concourse.tile as tile
from concourse import bacc
from concourse.bass_utils import run_bass_kernel_spmd
from concourse.masks import make_identity

N_CORES = 8
B, S, D, M = 32, 8192, 512, 16
S_MACRO = 512          # s rows per macro-tile
T_SUB = S_MACRO // 128  # 128-row subtiles per macro-tile
DC = D // 128           # 128-col d chunks

f32 = mybir.dt.float32
bf16 = mybir.dt.bfloat16


def kernel_body(tc, out_ap, x_ap, seeds_ap, b_loc, s):
    nc = tc.nc
    n_mac = s // S_MACRO
    with ExitStack() as ctx:
        const = ctx.enter_context(tc.tile_pool(name="const", bufs=1))
        xp = ctx.enter_context(tc.tile_pool(name="xp", bufs=7))
        xtp = ctx.enter_context(tc.tile_pool(name="xtp", bufs=4))
        ep = ctx.enter_context(tc.tile_pool(name="ep", bufs=4))
        etp = ctx.enter_context(tc.tile_pool(name="etp", bufs=4))
        statp = ctx.enter_context(tc.tile_pool(name="statp", bufs=4))
        outp = ctx.enter_context(tc.tile_pool(name="outp", bufs=2))
        ps_xt = ctx.enter_context(tc.tile_pool(name="ps_xt", bufs=4, space="PSUM"))
        ps_sc = ctx.enter_context(tc.tile_pool(name="ps_sc", bufs=1, space="PSUM"))
        ps_et = ctx.enter_context(tc.tile_pool(name="ps_et", bufs=1, space="PSUM"))
        ps_pl = ctx.enter_context(tc.tile_pool(name="ps_pl", bufs=2, space="PSUM"))

        ident = const.tile([128, 128], bf16)
        make_identity(nc, ident)

        # seeds -> bf16 -> seedsT [d, m] chunks, [128, DC*M] (dc-major)
        seeds_bf = const.tile([M, D], bf16)
        nc.gpsimd.dma_start(out=seeds_bf[:], in_=seeds_ap)
        ps_st = ps_et.tile([128, DC * M], bf16, tag="et", name="et")
        for dc in range(DC):
            nc.tensor.transpose(
                ps_st[:, dc * M:(dc + 1) * M],
                seeds_bf[:, dc * 128:(dc + 1) * 128],
                ident[:M, :M],
            )
        seedsT = const.tile([128, DC * M], bf16)
        nc.vector.tensor_copy(seedsT[:], ps_st[:])

        # x view: [b, n, p, q, d] with s = n*S_MACRO + p*T_SUB + q.
        # Partition p holds T_SUB consecutive s rows -> 8KB contiguous HBM
        # reads per partition (4x fewer DMA packets). The s-order inside a
        # macro is a fixed permutation; softmax is permutation-invariant and
        # scores/exp/pooled all use the same block mapping, so it cancels.
        x_r = x_ap.rearrange("b (n p q) d -> b n p q d", p=128, q=T_SUB)

        # Software-pipelined across all (batch, macro) pairs:
        #   stage A(i): DMA load, PE x-transposes, DVE psum->sbuf copies
        #   stage B(i): scores matmuls, ACT exp (+row-sum)
        #   stage C(i): PE exp-transposes, DVE copy, pooled matmuls, finalize
        # C runs 2 macros behind A so PE never waits on the ACT/DVE round
        # trip of the same macro.
        macros = [(bb, n) for bb in range(b_loc) for n in range(n_mac)]
        NM = len(macros)
        st = {}  # per-macro live tiles
        batch = {}  # per-batch state: sums tile, pool psum

        def stage_a(i):
            bb, n = macros[i]
            x_bf = xp.tile([128, T_SUB, D], bf16, tag="x", name="x_bf")
            nc.gpsimd.dma_start(out=x_bf[:], in_=x_r[bb, n])
            xt_sb = xtp.tile([128, DC, S_MACRO], bf16, tag="xt", name="xt")
            for ph in range(DC // 2):  # 2 dc chunks per psum bank
                xt_ps = ps_xt.tile([128, 2 * S_MACRO], bf16, tag="xt", name="xt")
                for dch in range(2):
                    dc = ph * 2 + dch
                    for t in range(T_SUB):
                        nc.tensor.transpose(
                            xt_ps[:, dch * S_MACRO + t * 128:
                                  dch * S_MACRO + (t + 1) * 128],
                            x_bf[:, t, dc * 128:(dc + 1) * 128],
                            ident[:],
                        )
                nc.vector.tensor_copy(xt_sb[:, ph * 2:(ph + 1) * 2, :], xt_ps[:])
            st[i] = {"x": x_bf, "xt": xt_sb}

        def stage_b(i):
            bb, n = macros[i]
            if n == 0:
                batch[bb] = {"sums": statp.tile([M, n_mac], f32, tag="sums", name="sums")}
            xt_sb = st[i]["xt"]
            sc_ps = ps_sc.tile([M, S_MACRO], f32, tag="sc", name="sc")
            for dc in range(DC):
                nc.tensor.matmul(
                    sc_ps[:],
                    lhsT=seedsT[:, dc * M:(dc + 1) * M],
                    rhs=xt_sb[:, dc, :],
                    start=(dc == 0),
                    stop=(dc == DC - 1),
                )
            e_bf = ep.tile([M, S_MACRO], bf16, tag="e", name="e_bf")
            nc.scalar.activation(
                e_bf[:], sc_ps[:], mybir.ActivationFunctionType.Exp,
                accum_out=batch[bb]["sums"][:, n:n + 1],
            )
            st[i]["e"] = e_bf

        def stage_c1(i):
            # expT transposes + small DVE copy; runs while scores(i+1) stream
            bb, n = macros[i]
            if n == 0:
                batch[bb]["pl"] = ps_pl.tile([128, D], f32, tag="pl", name="pl")
            e_bf = st[i]["e"]
            et_ps = ps_et.tile([128, T_SUB * M], bf16, tag="et", name="et")
            for t in range(T_SUB):
                nc.tensor.transpose(
                    et_ps[:, t * M:(t + 1) * M],
                    e_bf[:, t * 128:(t + 1) * 128],
                    ident[:M, :M],
                )
            et_sb = etp.tile([128, T_SUB * M], bf16, tag="et", name="et")
            nc.vector.tensor_copy(et_sb[:], et_ps[:])
            st[i]["et"] = et_sb

        def stage_c2(i):
            bb, n = macros[i]
            x_bf = st[i]["x"]
            et_sb = st[i]["et"]
            pool_ps = batch[bb]["pl"]
            # 4-way column-tiled: each q-block runs in its own 32-col group
            # of the PE array with its own XBUS stream; the 4 streams run
            # concurrently. Partial sums land on psum partitions 32q..32q+15
            # and are reduced once per batch.
            for t in range(T_SUB):
                nc.tensor.matmul(
                    pool_ps[32 * t:32 * t + M, :],
                    lhsT=et_sb[:, t * M:(t + 1) * M],
                    rhs=x_bf[:, t, :],
                    start=(n == 0),
                    stop=(n == n_mac - 1),
                    tile_position=(0, 32 * t),
                    skip_group_check=True,
                )
            del st[i]
            if n == n_mac - 1:
                sums = batch[bb]["sums"]
                total = statp.tile([M, 1], f32, tag="tot", name="tot")
                nc.vector.reduce_sum(total[:], sums[:], axis=mybir.AxisListType.X)
                recip = statp.tile([M, 1], f32, tag="rec", name="rec")
                nc.vector.reciprocal(recip[:], total[:])
                pa = statp.tile([M, D], f32, tag="pa", name="pa")
                nc.vector.tensor_copy(pa[:], pool_ps[0:M, :])
                nc.vector.tensor_add(pa[:], pa[:], pool_ps[32:32 + M, :])
                nc.vector.tensor_add(pa[:], pa[:], pool_ps[64:64 + M, :])
                nc.vector.tensor_add(pa[:], pa[:], pool_ps[96:96 + M, :])
                o_sb = outp.tile([M, D], f32, tag="o", name="o_sb")
                nc.vector.tensor_scalar_mul(o_sb[:], pa[:], recip[:])
                nc.sync.dma_start(out=out_ap[bb], in_=o_sb[:])
                del batch[bb]

        for i in range(NM + 2):
            if 2 <= i <= NM + 1:
                stage_c1(i - 2)
            if 1 <= i <= NM:
                stage_b(i - 1)
            if 2 <= i <= NM + 1:
                stage_c2(i - 2)
            if i < NM:
                stage_a(i)


def build_bass(b_loc, s):
    nc = bacc.Bacc(
        "TRN2", target_bir_lowering=False, debug=False, num_devices=N_CORES
    )
    x_d = nc.dram_tensor("x", [b_loc, s, D], f32, kind="ExternalInput")
    seeds_d = nc.dram_tensor("seeds", [M, D], f32, kind="ExternalInput")
    out_d = nc.dram_tensor("out", [b_loc, M, D], f32, kind="ExternalOutput")
    with tile.TileContext(nc) as tc:
        kernel_body(tc, out_d.ap(), x_d.ap(), seeds_d.ap(), b_loc, s)
    nc.compile()
    return nc


_cached = {}


def get_nc(b_loc, s):
    key = (b_loc, s)
    if key not in _cached:
        _cached[key] = build_bass(b_loc, s)
    return _cached[key]


def kernel(x, seeds, trace=False):
    assert x.shape == (B, S, D) and seeds.shape == (M, D)
    x = np.asarray(x, dtype=np.float32)
    seeds = np.asarray(seeds, dtype=np.float32)
    b_loc = B // N_CORES
    nc = get_nc(b_loc, S)
    in_maps = [
        {
            "x": np.ascontiguousarray(x[i * b_loc:(i + 1) * b_loc]),
            "seeds": seeds,
        }
        for i in range(N_CORES)
    ]
    res = run_bass_kernel_spmd(
        nc, in_maps, core_ids=list(range(N_CORES)), trace=trace
    )
    out = np.concatenate([r["out"] for r in res.results], axis=0)
    if trace:
        kernel.last_result = res
    return out.astype(np.float32)


kernel.last_result = None


# revision 34
# speedup vs baseline: 1.0809x; 1.0809x over previous
"""Attention-pooling kernel for TRN2 (8 NeuronCores, data-parallel over batch).

Computes, per batch b:
    scores = seeds @ x[b].T          # [M, S]
    weights = softmax(scores, -1)
    out[b] = weights @ x[b]          # [M, D]

Sharding: batch B=32 split 4-per-core across 8 cores; seeds replicated.

Per-core pipeline (all bf16 on-chip, f32 PSUM accumulation):
  - SWDGE cast-DMA loads x tiles HBM f32 -> SBUF bf16 (cast rides the DMA).
  - PE transposes x 128x128 blocks (bf16, FWL weight loads) -> psum -> DVE
    copies to SBUF as x^T chunks.
  - scores: 4 accumulating matmuls lhsT=seedsT chunk [128,16], rhs=xT [128,512].
  - exp on ACT straight out of PSUM, with fused accum_out row-sums
    (no max subtraction: scores = seeds.x are bounded ~|8|, exp is safe in f32).
  - PE transposes exp [16,128] -> expT [128,16]; pooled matmuls are 4-way
    column-tiled (concurrent PE col-groups), partials accumulate in
    psum[32q:32q+16, :] over the whole batch.
  - batch end: reduce partials + recip(sum) on DVE, scale, DMA out f32.
  - Stages are software-pipelined (C(i-2), B(i-1), A(i)) so PE never waits
    on the ACT/DVE round trip of the same macro-tile.
"""

from contextlib import ExitStack

import numpy as np

import concourse.mybir as mybir
import concourse.tile as tile
from concourse import bacc
from concourse.bass_utils import run_bass_kernel_spmd
from concourse.masks import make_identity

N_CORES = 8
B, S, D, M = 32, 8192, 512, 16
S_MACRO = 512          # s rows per macro-tile
T_SUB = S_MACRO // 128  # 128-row subtiles per macro-tile
DC = D // 128           # 128-col d chunks

f32 = mybir.dt.float32
bf16 = mybir.dt.bfloat16


def kernel_body(tc, out_ap, x_ap, seeds_ap, b_loc, s):
    nc = tc.nc
    n_mac = s // S_MACRO
    with ExitStack() as ctx:
        const = ctx.enter_context(tc.tile_pool(name="const", bufs=1))
        xp = ctx.enter_context(tc.tile_pool(name="xp", bufs=7))
        xtp = ctx.enter_context(tc.tile_pool(name="xtp", bufs=4))
        ep = ctx.enter_context(tc.tile_pool(name="ep", bufs=4))
        etp = ctx.enter_context(tc.tile_pool(name="etp", bufs=4))
        statp = ctx.enter_context(tc.tile_pool(name="statp", bufs=4))
        outp = ctx.enter_context(tc.tile_pool(name="outp", bufs=2))
        ps_xt = ctx.enter_context(tc.tile_pool(name="ps_xt", bufs=4, space="PSUM"))
        ps_sc = ctx.enter_context(tc.tile_pool(name="ps_sc", bufs=1, space="PSUM"))
        ps_et = ctx.enter_context(tc.tile_pool(name="ps_et", bufs=1, space="PSUM"))
        ps_pl = ctx.enter_context(tc.tile_pool(name="ps_pl", bufs=2, space="PSUM"))

        ident = const.tile([128, 128], bf16)
        make_identity(nc, ident)

        # seeds -> bf16 -> seedsT [d, m] chunks, [128, DC*M] (dc-major)
        seeds_bf = const.tile([M, D], bf16)
        nc.gpsimd.dma_start(out=seeds_bf[:], in_=seeds_ap)
        ps_st = ps_et.tile([128, DC * M], bf16, tag="et", name="et")
        for dc in range(DC):
            nc.tensor.transpose(
                ps_st[:, dc * M:(dc + 1) * M],
                seeds_bf[:, dc * 128:(dc + 1) * 128],
                ident[:M, :M],
            )
        seedsT = const.tile([128, DC * M], bf16)
        nc.vector.tensor_copy(seedsT[:], ps_st[:])

        # x view: [b, n, p, q, d] with s = n*S_MACRO + p*T_SUB + q.
        # Partition p holds T_SUB consecutive s rows -> 8KB contiguous HBM
        # reads per partition (4x fewer DMA packets). The s-order inside a
        # macro is a fixed permutation; softmax is permutation-invariant and
        # scores/exp/pooled all use the same block mapping, so it cancels.
        x_r = x_ap.rearrange("b (n p q) d -> b n p q d", p=128, q=T_SUB)

        # Software-pipelined across all (batch, macro) pairs:
        #   stage A(i): DMA load, PE x-transposes, DVE psum->sbuf copies
        #   stage B(i): scores matmuls, ACT exp (+row-sum)
        #   stage C(i): PE exp-transposes, DVE copy, pooled matmuls, finalize
        # C runs 2 macros behind A so PE never waits on the ACT/DVE round
        # trip of the same macro.
        macros = [(bb, n) for bb in range(b_loc) for n in range(n_mac)]
        NM = len(macros)
        st = {}  # per-macro live tiles
        batch = {}  # per-batch state: sums tile, pool psum

        def stage_a(i):
            bb, n = macros[i]
            x_bf = xp.tile([128, T_SUB, D], bf16, tag="x", name="x_bf")
            nc.gpsimd.dma_start(out=x_bf[:], in_=x_r[bb, n])
            xt_sb = xtp.tile([128, DC, S_MACRO], bf16, tag="xt", name="xt")
            for ph in range(DC // 2):  # 2 dc chunks per psum bank
                xt_ps = ps_xt.tile([128, 2 * S_MACRO], bf16, tag="xt", name="xt")
                for dch in range(2):
                    dc = ph * 2 + dch
                    for t in range(T_SUB):
                        nc.tensor.transpose(
                            xt_ps[:, dch * S_MACRO + t * 128:
                                  dch * S_MACRO + (t + 1) * 128],
                            x_bf[:, t, dc * 128:(dc + 1) * 128],
                            ident[:],
                        )
                nc.vector.tensor_copy(xt_sb[:, ph * 2:(ph + 1) * 2, :], xt_ps[:])
            st[i] = {"x": x_bf, "xt": xt_sb}

        def stage_b(i):
            bb, n = macros[i]
            if n == 0:
                batch[bb] = {"sums": statp.tile([M, n_mac], f32, tag="sums", name="sums")}
            xt_sb = st[i]["xt"]
            sc_ps = ps_sc.tile([M, S_MACRO], f32, tag="sc", name="sc")
            for dc in range(DC):
                nc.tensor.matmul(
                    sc_ps[:],
                    lhsT=seedsT[:, dc * M:(dc + 1) * M],
                    rhs=xt_sb[:, dc, :],
                    start=(dc == 0),
                    stop=(dc == DC - 1),
                )
            e_bf = ep.tile([M, S_MACRO], bf16, tag="e", name="e_bf")
            nc.scalar.activation(
                e_bf[:], sc_ps[:], mybir.ActivationFunctionType.Exp,
                accum_out=batch[bb]["sums"][:, n:n + 1],
            )
            st[i]["e"] = e_bf

        def stage_c1(i):
            # expT transposes + small DVE copy; runs while scores(i+1) stream
            bb, n = macros[i]
            if n == 0:
                batch[bb]["pl"] = ps_pl.tile([128, D], f32, tag="pl", name="pl")
            e_bf = st[i]["e"]
            et_ps = ps_et.tile([128, T_SUB * M], bf16, tag="et", name="et")
            for t in range(T_SUB):
                nc.tensor.transpose(
                    et_ps[:, t * M:(t + 1) * M],
                    e_bf[:, t * 128:(t + 1) * 128],
                    ident[:M, :M],
                )
            et_sb = etp.tile([128, T_SUB * M], bf16, tag="et", name="et")
            nc.vector.tensor_copy(et_sb[:], et_ps[:])
            st[i]["et"] = et_sb

        def stage_c2(i):
            bb, n = macros[i]
            x_bf = st[i]["x"]
            et_sb = st[i]["et"]
            pool_ps = batch[bb]["pl"]
            # 4-way column-tiled: each q-block runs in its own 32-col group
            # of the PE array with its own XBUS stream; the 4 streams run
            # concurrently. Partial sums land on psum partitions 32q..32q+15
            # and are reduced once per batch.
            for t in range(T_SUB):
                nc.tensor.matmul(
                    pool_ps[32 * t:32 * t + M, :],
                    lhsT=et_sb[:, t * M:(t + 1) * M],
                    rhs=x_bf[:, t, :],
                    start=(n == 0),
                    stop=(n == n_mac - 1),
                    tile_position=(0, 32 * t),
                    skip_group_check=True,
                )
            del st[i]
            if n == n_mac - 1:
                sums = batch[bb]["sums"]
                total = statp.tile([M, 1], f32, tag="tot", name="tot")
                nc.vector.reduce_sum(total[:], sums[:], axis=mybir.AxisListType.X)
                recip = statp.tile([M, 1], f32, tag="rec", name="rec")
                nc.vector.reciprocal(recip[:], total[:])
                # fused: o = sum_q partial_q * recip, one op per partial
                o_sb = outp.tile([M, D], f32, tag="o", name="o_sb")
                nc.vector.tensor_scalar_mul(o_sb[:], pool_ps[0:M, :], recip[:])
                for q in range(1, T_SUB):
                    nc.vector.scalar_tensor_tensor(
                        o_sb[:], pool_ps[32 * q:32 * q + M, :], recip[:],
                        o_sb[:], op0=mybir.AluOpType.mult,
                        op1=mybir.AluOpType.add,
                    )
                nc.sync.dma_start(out=out_ap[bb], in_=o_sb[:])
                del batch[bb]

        for i in range(NM + 2):
            if 2 <= i <= NM + 1:
                stage_c1(i - 2)
            if 1 <= i <= NM:
                stage_b(i - 1)
            if 2 <= i <= NM + 1:
                stage_c2(i - 2)
            if i < NM:
                stage_a(i)


def build_bass(b_loc, s):
    nc = bacc.Bacc(
        "TRN2", target_bir_lowering=False, debug=False, num_devices=N_CORES
    )
    x_d = nc.dram_tensor("x", [b_loc, s, D], f32, kind="ExternalInput")
    seeds_d = nc.dram_tensor("seeds", [M, D], f32, kind="ExternalInput")
    out_d = nc.dram_tensor("out", [b_loc, M, D], f32, kind="ExternalOutput")
    with tile.TileContext(nc) as tc:
        kernel_body(tc, out_d.ap(), x_d.ap(), seeds_d.ap(), b_loc, s)
    nc.compile()
    return nc


_cached = {}


def get_nc(b_loc, s):
    key = (b_loc, s)
    if key not in _cached:
        _cached[key] = build_bass(b_loc, s)
    return _cached[key]


def kernel(x, seeds, trace=False):
    assert x.shape == (B, S, D) and seeds.shape == (M, D)
    x = np.asarray(x, dtype=np.float32)
    seeds = np.asarray(seeds, dtype=np.float32)
    b_loc = B // N_CORES
    nc = get_nc(b_loc, S)
    in_maps = [
        {
            "x": np.ascontiguousarray(x[i * b_loc:(i + 1) * b_loc]),
            "seeds": seeds,
        }
        for i in range(N_CORES)
    ]
    res = run_bass_kernel_spmd(
        nc, in_maps, core_ids=list(range(N_CORES)), trace=trace
    )
    out = np.concatenate([r["out"] for r in res.results], axis=0)
    if trace:
        kernel.last_result = res
    return out.astype(np.float32)


kernel.last_result = None
